# revision 26
# baseline (speedup 1.0000x reference)
"""Distributed GQA attention kernel for Trainium2 (8 NeuronCores).

Sharding: 2-way data parallel over batch x 4-way tensor parallel over heads.
Core c handles batch b = c // 4 and head group g = c % 4 (8 q-heads, 2 kv-heads).
Each core computes a full-size partial of the output (its head group pushed
through Wo); the host sums the 4 partials per batch. No on-device collective.

Device-side layout is feature-major (Q^T/K^T: [feature partitions, T free]) so
projections consume the host-pre-transposed x^T directly, attention scores are
computed transposed (S^T[tk, tq]) so softmax(P)@V needs no transposes, and the
softmax denominator comes free from an appended ones-column on V.
"""

import numpy as np
import ml_dtypes
from contextlib import ExitStack

import concourse.bass as bass
from concourse import bacc
import concourse.mybir as mybir
import concourse.tile as tile
from concourse.bass_utils import run_bass_kernel_spmd

BF16 = mybir.dt.bfloat16
F32 = mybir.dt.float32
AF = mybir.ActivationFunctionType

P = 128
B, T, D = 2, 2048, 2048
NUM_HEADS, NUM_KV_HEADS, HD = 32, 8, 64
FQ = 512          # q features per core (8 heads x 64)
DKV = 128         # kv features per core (2 kv heads x 64)
KO = D // P       # 16 contraction tiles over d_model
NT = T // 512     # 4 tiles of 512 along T
SCALE = 1.0 / np.sqrt(HD)
ROPE_BASE = 10000.0
# local head order inside the 512 q-features: pairs (j, j+4) so that the two
# heads in partition tile j sit at bases 0/64 matching kv heads 0/1 in K^T
PERM_Q = [0, 4, 1, 5, 2, 6, 3, 7]

_nc_cache = {}


def build_nc():
    if "nc" in _nc_cache:
        return _nc_cache["nc"]
    nc = bacc.Bacc()
    xT = nc.declare_dram_parameter("xT", [D, T], BF16, isOutput=False)
    wq = nc.declare_dram_parameter("wqT", [D, FQ], BF16, isOutput=False)
    wk = nc.declare_dram_parameter("wkT", [D, DKV], BF16, isOutput=False)
    wv = nc.declare_dram_parameter("wvT", [D, DKV], BF16, isOutput=False)
    wo = nc.declare_dram_parameter("woT", [FQ, D], BF16, isOutput=False)
    cosd = nc.declare_dram_parameter("cosT", [P, T], BF16, isOutput=False)
    sind = nc.declare_dram_parameter("sinT", [P, T], BF16, isOutput=False)
    mskd = nc.declare_dram_parameter("tri", [P, P], BF16, isOutput=False)
    y = nc.declare_dram_parameter("y", [T, D], F32, isOutput=True)

    with tile.TileContext(nc) as tc:
        with ExitStack() as ctx:
            const = ctx.enter_context(tc.tile_pool(name="const", bufs=1))
            work = ctx.enter_context(tc.tile_pool(name="work", bufs=3))
            otp = ctx.enter_context(tc.tile_pool(name="otp", bufs=2))
            pexp = ctx.enter_context(tc.tile_pool(name="pexp", bufs=8))
            rrp = ctx.enter_context(tc.tile_pool(name="rrp", bufs=2))
            dramp = ctx.enter_context(tc.tile_pool(name="dramp", bufs=2, space="DRAM"))
            big_ps = ctx.enter_context(tc.tile_pool(name="bigps", bufs=1, space="PSUM"))
            pv_ps = ctx.enter_context(tc.tile_pool(name="pvps", bufs=2, space="PSUM"))
            s_ps = ctx.enter_context(tc.tile_pool(name="sps", bufs=5, space="PSUM"))

            # ---- constant loads ----
            # small early-needed constants first, then x (K/V proj gate on
            # it), then wq; wo is deferred off the startup DMA critical path
            cos_sb = const.tile([P, T], BF16, tag="cos")
            sin_sb = const.tile([P, T], BF16, tag="sin")
            nc.sync.dma_start(cos_sb[:], cosd[:])
            nc.sync.dma_start(sin_sb[:], sind[:])
            tri_sb = const.tile([P, P], BF16, tag="tri")
            nc.sync.dma_start(tri_sb[:], mskd[:])
            wk_sb = const.tile([P, KO, DKV], BF16, tag="wk")
            wv_sb = const.tile([P, KO, DKV], BF16, tag="wv")
            for ko in range(KO):
                nc.sync.dma_start(wk_sb[:, ko, :], wk[ko * P:(ko + 1) * P, :])
                nc.sync.dma_start(wv_sb[:, ko, :], wv[ko * P:(ko + 1) * P, :])
            x_sb = const.tile([P, KO, T], BF16, tag="x")
            for ko in range(KO):
                nc.sync.dma_start(x_sb[:, ko, :], xT[ko * P:(ko + 1) * P, :])
            wq_sb = const.tile([P, KO, FQ], BF16, tag="wq")
            for ko in range(KO):
                nc.sync.dma_start(wq_sb[:, ko, :], wq[ko * P:(ko + 1) * P, :])
            wo_sb = const.tile([P, 4, D], BF16, tag="wo")

            def rope(dst_ap, ps, nt, tag):
                """cast psum->bf16, rotate halves, combine with cos/sin tables"""
                raw = work.tile([P, 512], BF16, tag="ropraw")
                nc.scalar.copy(raw[:], ps[:])
                rot = work.tile([P, 512], BF16, tag="roprot")
                for h in range(2):
                    b0 = h * 64
                    nc.sync.dma_start(rot[b0:b0 + 32, :], raw[b0 + 32:b0 + 64, :])
                    nc.sync.dma_start(rot[b0 + 32:b0 + 64, :], raw[b0:b0 + 32, :])
                ts = slice(nt * 512, (nt + 1) * 512)
                t1 = work.tile([P, 512], BF16, tag="ropt1")
                nc.vector.tensor_mul(t1[:], raw[:], cos_sb[:, ts])
                nc.vector.tensor_mul(rot[:], rot[:], sin_sb[:, ts])
                nc.vector.tensor_add(dst_ap, t1[:], rot[:])

            # ---- K projection + rope (feature-major K^T [128, T]) ----
            kt = const.tile([P, T], BF16, tag="kt")
            for nt in range(NT):
                ps = big_ps.tile([P, 512], F32, tag="big")
                for ko in range(KO):
                    nc.tensor.matmul(ps[:], wk_sb[:, ko, :],
                                     x_sb[:, ko, nt * 512:(nt + 1) * 512],
                                     start=(ko == 0), stop=(ko == KO - 1))
                rope(kt[:, nt * 512:(nt + 1) * 512], ps, nt, "k")

            # ---- V projection (token-major, with ones column appended) ----
            # v_sb[:, tt, 0:65] = [V_kv0 | 1], v_sb[:, tt, 65:130] = [V_kv1 | 1]
            v_sb = const.tile([P, 16, 130], BF16, tag="v")
            nc.gpsimd.memset(v_sb[:, :, 64:65], 1.0)
            nc.gpsimd.memset(v_sb[:, :, 129:130], 1.0)
            for tt in range(16):
                ps = big_ps.tile([P, DKV], F32, tag="big")
                for ko in range(KO):
                    nc.tensor.matmul(ps[:], x_sb[:, ko, tt * P:(tt + 1) * P],
                                     wv_sb[:, ko, :],
                                     start=(ko == 0), stop=(ko == KO - 1))
                nc.vector.tensor_copy(v_sb[:, tt, 0:64], ps[:, 0:64])
                nc.vector.tensor_copy(v_sb[:, tt, 65:129], ps[:, 64:128])

            # ---- Q projection + rope for one head pair ----
            qts = {}

            def q_proj(j):
                qt_j = const.tile([P, T], BF16, tag=f"qt{j}")
                for nt in range(NT):
                    ps = big_ps.tile([P, 512], F32, tag="big")
                    for ko in range(KO):
                        nc.tensor.matmul(ps[:], wq_sb[:, ko, j * P:(j + 1) * P],
                                         x_sb[:, ko, nt * 512:(nt + 1) * 512],
                                         start=(ko == 0), stop=(ko == KO - 1))
                    rope(qt_j[:, nt * 512:(nt + 1) * 512], ps, nt, f"q{j}")
                qts[j] = qt_j

            # ---- attention for one (qt, j) head-pair into ot tile ----
            def attn_block(qt, j, ot):
                pv0 = pv_ps.tile([65, 512], F32, tag="pv")
                pv1 = pv_ps.tile([65, 512], F32, tag="pv")
                nkb = 4 * qt + 4

                def flush_pv(prev):
                    # PV matmuls for the previous kb (software pipeline: issued
                    # after the next kb's scores so PE never waits on ACT's exp
                    # of the current block). Diagonal blocks only touch output
                    # columns >= their first causally-valid query.
                    pkb, c0, q0, q1 = prev
                    nc.tensor.matmul(pv0[:, c0:512], v_sb[:, pkb, 0:65],
                                     q0[:, c0:512],
                                     start=(pkb == 0), stop=(pkb == nkb - 1))
                    nc.tensor.matmul(pv1[:, c0:512], v_sb[:, pkb, 65:130],
                                     q1[:, c0:512],
                                     start=(pkb == 0), stop=(pkb == nkb - 1))

                prev = None
                for kb in range(nkb):
                    tk = slice(kb * P, (kb + 1) * P)
                    jr = kb - 4 * qt           # >= 0 on diagonal blocks
                    c0 = max(0, jr) * P        # first causally-valid column
                    tqs = slice(qt * 512 + c0, (qt + 1) * 512)
                    s0 = s_ps.tile([P, 512], F32, tag="s")
                    s1 = s_ps.tile([P, 512], F32, tag="s")
                    nc.tensor.matmul(s0[:, c0:512], kt[0:64, tk],
                                     qts[j][0:64, tqs], start=True, stop=True)
                    nc.tensor.matmul(s1[:, c0:512], kt[64:128, tk],
                                     qts[j][64:128, tqs], start=True, stop=True)
                    if prev is not None:
                        flush_pv(prev)
                    p0 = pexp.tile([P, 512], BF16, tag="p")
                    p1 = pexp.tile([P, 512], BF16, tag="p")
                    nc.scalar.activation(p0[:, c0:512], s0[:, c0:512],
                                         AF.Exp, scale=SCALE)
                    nc.scalar.activation(p1[:, c0:512], s1[:, c0:512],
                                         AF.Exp, scale=SCALE)
                    if jr >= 0:
                        # triangle mask on the one partially-valid block
                        nc.vector.tensor_mul(p0[:, c0:c0 + P],
                                             p0[:, c0:c0 + P], tri_sb[:])
                        nc.vector.tensor_mul(p1[:, c0:c0 + P],
                                             p1[:, c0:c0 + P], tri_sb[:])
                    prev = (kb, c0, p0, p1)
                flush_pv(prev)
                # fast pv release: stage numerator + sumexp row, then the
                # recip/broadcast/mul chain runs off the pv critical path
                for idx, pv in ((0, pv0), (1, pv1)):
                    osl = ot[idx * 64:(idx + 1) * 64, j, :]
                    nc.vector.tensor_copy(osl, pv[0:64, :])
                    srow = rrp.tile([1, 512], F32, tag="sr")
                    nc.vector.tensor_copy(srow[:], pv[64:65, :])
                    rrow = rrp.tile([1, 512], F32, tag="rr")
                    nc.vector.reciprocal_approx_fast(rrow[:], srow[:])
                    drx = dramp.tile([1, 512], F32, tag="drx")
                    nc.sync.dma_start(drx[:], rrow[:])
                    bc = rrp.tile([P, 512], F32, tag="bc")
                    bsl = bc[idx * 64:(idx + 1) * 64, :]
                    nc.sync.dma_start(bsl, drx[:].to_broadcast((64, 512)))
                    nc.vector.tensor_mul(osl, osl, bsl)

            # ---- Wo output projection for one 128-row slice of a q tile ----
            def wo_block(qt, tt, ot):
                r0 = qt * 512 + tt * P
                for oc in range(4):
                    yps = big_ps.tile([P, 512], F32, tag="big")
                    for kf in range(4):
                        nc.tensor.matmul(yps[:], ot[:, kf, tt * P:(tt + 1) * P],
                                         wo_sb[:, kf, oc * 512:(oc + 1) * 512],
                                         start=(kf == 0), stop=(kf == 3))
                    ysb = work.tile([P, 512], F32, tag="ysb")
                    nc.vector.tensor_copy(ysb[:], yps[:])
                    nc.sync.dma_start(y[r0:r0 + P, oc * 512:(oc + 1) * 512], ysb[:])

            # ---- emission order: interleave phases so PE work (proj / Wo)
            # fills the gaps while ACT runs exp of the attention stream ----
            ot_tiles = {0: otp.tile([P, 4, 512], BF16, tag="ot", name="ot0")}
            for j in range(4):
                q_proj(j)
                if j == 1:
                    for kf in range(4):
                        nc.sync.dma_start(wo_sb[:, kf, :], wo[kf * P:(kf + 1) * P, :])
                attn_block(0, j, ot_tiles[0])      # qt=0 rides inside Q proj
            for qt in range(1, NT):
                ot_tiles[qt] = otp.tile([P, 4, 512], BF16, tag="ot", name=f"ot{qt}")
                for j in range(4):
                    attn_block(qt, j, ot_tiles[qt])
                    wo_block(qt - 1, j, ot_tiles[qt - 1])  # prev qt's Wo rides along
            for tt in range(4):
                wo_block(NT - 1, tt, ot_tiles[NT - 1])

    nc.finalize()
    _nc_cache["nc"] = nc
    return nc


def make_in_maps(x, Wq, Wk, Wv, Wo):
    bf = ml_dtypes.bfloat16
    x = np.asarray(x, np.float32)
    Wq = np.asarray(Wq, np.float32)
    Wk = np.asarray(Wk, np.float32)
    Wv = np.asarray(Wv, np.float32)
    Wo = np.asarray(Wo, np.float32)

    # rope tables, [128, T]: row p covers head-dim d = p % 64
    half = HD // 2
    inv_freq = 1.0 / (ROPE_BASE ** (np.arange(half, dtype=np.float64) / half))
    pos = np.arange(T, dtype=np.float64)
    d_idx = np.arange(P) % HD
    freqs = pos[None, :] * inv_freq[d_idx % half][:, None]      # [128, T]
    cos_t = np.cos(freqs).astype(bf)
    sign = np.where(d_idx < half, -1.0, 1.0)[:, None]
    sin_t = (np.sin(freqs) * sign).astype(bf)

    # causal 0/1 triangle for the partially-valid diagonal sub-block
    pp = np.arange(P)[:, None]
    ff = np.arange(P)[None, :]
    tri = (ff >= pp).astype(bf)

    in_maps = []
    for c in range(8):
        b, g = c // 4, c % 4
        heads = [8 * g + h for h in PERM_Q]
        qrows = np.concatenate([np.arange(h * HD, (h + 1) * HD) for h in heads])
        kvrows = np.arange(2 * g * HD, (2 * g + 2) * HD)
        in_maps.append({
            "xT": np.ascontiguousarray(x[b].T).astype(bf),
            "wqT": np.ascontiguousarray(Wq[qrows, :].T).astype(bf),
            "wkT": np.ascontiguousarray(Wk[kvrows, :].T).astype(bf),
            "wvT": np.ascontiguousarray(Wv[kvrows, :].T).astype(bf),
            "woT": np.ascontiguousarray(Wo[:, qrows].T).astype(bf),
            "cosT": cos_t,
            "sinT": sin_t,
            "tri": tri,
        })
    return in_maps


def combine_outputs(results):
    out = np.zeros((B, T, D), np.float32)
    for c in range(8):
        out[c // 4] += results[c]["y"]
    return out


def _ensure_ntff_hook():
    """Register the axon NTFF profile hook (antenv.axon_hooks is missing
    from this image; recreate it and wire the ctypes hook from trn_boot)."""
    import sys, types
    if "antenv.axon_hooks" in sys.modules:
        return
    m = types.ModuleType("antenv.axon_hooks")
    hook = [None]
    m.set_axon_ntff_profile_hook = lambda h: hook.__setitem__(0, h)
    m.get_axon_ntff_profile_hook = lambda: hook[0]
    sys.modules["antenv.axon_hooks"] = m
    import antenv
    antenv.axon_hooks = m
    sys.path.insert(0, "/root/.axon_site")
    from trn_agent_boot.trn_boot import _ntff_profile_via_ctypes
    m.set_axon_ntff_profile_hook(
        _ntff_profile_via_ctypes("/opt/axon/libaxon_pjrt.so"))


def kernel(x, Wq, Wk, Wv, Wo, _trace=False):
    if _trace:
        _ensure_ntff_hook()
    nc = build_nc()
    in_maps = make_in_maps(x, Wq, Wk, Wv, Wo)
    res = run_bass_kernel_spmd(nc, in_maps, core_ids=list(range(8)), trace=_trace)
    out = combine_outputs(res.results)
    if _trace:
        return out, res
    return out


# revision 27
# speedup vs baseline: 1.0794x; 1.0794x over previous
"""Distributed GQA attention kernel for Trainium2 (8 NeuronCores).

Sharding: 2-way data parallel over batch x 4-way tensor parallel over heads.
Core c handles batch b = c // 4 and head group g = c % 4 (8 q-heads, 2 kv-heads).
Each core computes a full-size partial of the output (its head group pushed
through Wo); the host sums the 4 partials per batch. No on-device collective.

Device-side layout is feature-major (Q^T/K^T: [feature partitions, T free]) so
projections consume the host-pre-transposed x^T directly, attention scores are
computed transposed (S^T[tk, tq]) so softmax(P)@V needs no transposes, and the
softmax denominator comes free from an appended ones-column on V.
"""

import numpy as np
import ml_dtypes
from contextlib import ExitStack

import concourse.bass as bass
from concourse import bacc
import concourse.mybir as mybir
import concourse.tile as tile
from concourse.bass_utils import run_bass_kernel_spmd

BF16 = mybir.dt.bfloat16
F32 = mybir.dt.float32
AF = mybir.ActivationFunctionType

P = 128
B, T, D = 2, 2048, 2048
NUM_HEADS, NUM_KV_HEADS, HD = 32, 8, 64
FQ = 512          # q features per core (8 heads x 64)
DKV = 128         # kv features per core (2 kv heads x 64)
KO = D // P       # 16 contraction tiles over d_model
NT = T // 512     # 4 tiles of 512 along T
SCALE = 1.0 / np.sqrt(HD)
ROPE_BASE = 10000.0
# local head order inside the 512 q-features: pairs (j, j+4) so that the two
# heads in partition tile j sit at bases 0/64 matching kv heads 0/1 in K^T
PERM_Q = [0, 4, 1, 5, 2, 6, 3, 7]

_nc_cache = {}


def build_nc():
    if "nc" in _nc_cache:
        return _nc_cache["nc"]
    nc = bacc.Bacc()
    xT = nc.declare_dram_parameter("xT", [D, T], BF16, isOutput=False)
    wq = nc.declare_dram_parameter("wqT", [D, FQ], BF16, isOutput=False)
    wk = nc.declare_dram_parameter("wkT", [D, DKV], BF16, isOutput=False)
    wv = nc.declare_dram_parameter("wvT", [D, DKV], BF16, isOutput=False)
    wo = nc.declare_dram_parameter("woT", [FQ, D], BF16, isOutput=False)
    cosd = nc.declare_dram_parameter("cosT", [P, T], BF16, isOutput=False)
    sind = nc.declare_dram_parameter("sinT", [P, T], BF16, isOutput=False)
    mskd = nc.declare_dram_parameter("tri", [P, P], BF16, isOutput=False)
    y = nc.declare_dram_parameter("y", [T, D], F32, isOutput=True)

    with tile.TileContext(nc) as tc:
        with ExitStack() as ctx:
            const = ctx.enter_context(tc.tile_pool(name="const", bufs=1))
            work = ctx.enter_context(tc.tile_pool(name="work", bufs=3))
            otp = ctx.enter_context(tc.tile_pool(name="otp", bufs=2))
            pexp = ctx.enter_context(tc.tile_pool(name="pexp", bufs=8))
            rrp = ctx.enter_context(tc.tile_pool(name="rrp", bufs=2))
            dramp = ctx.enter_context(tc.tile_pool(name="dramp", bufs=2, space="DRAM"))
            big_ps = ctx.enter_context(tc.tile_pool(name="bigps", bufs=2, space="PSUM"))
            pv_ps = ctx.enter_context(tc.tile_pool(name="pvps", bufs=2, space="PSUM"))
            s_ps = ctx.enter_context(tc.tile_pool(name="sps", bufs=4, space="PSUM"))

            # ---- constant loads ----
            # small early-needed constants first, then x (K/V proj gate on
            # it), then wq; wo is deferred off the startup DMA critical path
            cos_sb = const.tile([P, T], BF16, tag="cos")
            sin_sb = const.tile([P, T], BF16, tag="sin")
            nc.sync.dma_start(cos_sb[:], cosd[:])
            nc.sync.dma_start(sin_sb[:], sind[:])
            tri_sb = const.tile([P, P], BF16, tag="tri")
            nc.sync.dma_start(tri_sb[:], mskd[:])
            wk_sb = const.tile([P, KO, DKV], BF16, tag="wk")
            wv_sb = const.tile([P, KO, DKV], BF16, tag="wv")
            for ko in range(KO):
                nc.sync.dma_start(wk_sb[:, ko, :], wk[ko * P:(ko + 1) * P, :])
                nc.sync.dma_start(wv_sb[:, ko, :], wv[ko * P:(ko + 1) * P, :])
            x_sb = const.tile([P, KO, T], BF16, tag="x")
            for ko in range(KO):
                nc.sync.dma_start(x_sb[:, ko, :], xT[ko * P:(ko + 1) * P, :])
            wq_sb = const.tile([P, KO, FQ], BF16, tag="wq")
            for ko in range(KO):
                nc.sync.dma_start(wq_sb[:, ko, :], wq[ko * P:(ko + 1) * P, :])
            wo_sb = const.tile([P, 4, D], BF16, tag="wo")

            def rope(dst_ap, ps, nt, tag):
                """cast psum->bf16, rotate halves, combine with cos/sin tables"""
                raw = work.tile([P, 512], BF16, tag="ropraw")
                nc.scalar.copy(raw[:], ps[:])
                rot = work.tile([P, 512], BF16, tag="roprot")
                for h in range(2):
                    b0 = h * 64
                    nc.sync.dma_start(rot[b0:b0 + 32, :], raw[b0 + 32:b0 + 64, :])
                    nc.sync.dma_start(rot[b0 + 32:b0 + 64, :], raw[b0:b0 + 32, :])
                ts = slice(nt * 512, (nt + 1) * 512)
                t1 = work.tile([P, 512], BF16, tag="ropt1")
                nc.vector.tensor_mul(t1[:], raw[:], cos_sb[:, ts])
                nc.vector.tensor_mul(rot[:], rot[:], sin_sb[:, ts])
                nc.vector.tensor_add(dst_ap, t1[:], rot[:])

            # ---- K projection + rope (feature-major K^T [128, T]) ----
            kt = const.tile([P, T], BF16, tag="kt")
            for nt in range(NT):
                ps = big_ps.tile([P, 512], F32, tag="big")
                for ko in range(KO):
                    nc.tensor.matmul(ps[:], wk_sb[:, ko, :],
                                     x_sb[:, ko, nt * 512:(nt + 1) * 512],
                                     start=(ko == 0), stop=(ko == KO - 1))
                rope(kt[:, nt * 512:(nt + 1) * 512], ps, nt, "k")

            # ---- V projection (token-major, with ones column appended) ----
            # v_sb[:, tt, 0:65] = [V_kv0 | 1], v_sb[:, tt, 65:130] = [V_kv1 | 1]
            v_sb = const.tile([P, 16, 130], BF16, tag="v")
            nc.gpsimd.memset(v_sb[:, :, 64:65], 1.0)
            nc.gpsimd.memset(v_sb[:, :, 129:130], 1.0)
            for tt in range(16):
                ps = big_ps.tile([P, DKV], F32, tag="big")
                for ko in range(KO):
                    nc.tensor.matmul(ps[:], x_sb[:, ko, tt * P:(tt + 1) * P],
                                     wv_sb[:, ko, :],
                                     start=(ko == 0), stop=(ko == KO - 1))
                nc.vector.tensor_copy(v_sb[:, tt, 0:64], ps[:, 0:64])
                nc.vector.tensor_copy(v_sb[:, tt, 65:129], ps[:, 64:128])

            # ---- Q projection + rope for one head pair ----
            qts = {}

            def q_proj(j):
                qt_j = const.tile([P, T], BF16, tag=f"qt{j}")
                for nt in range(NT):
                    ps = big_ps.tile([P, 512], F32, tag="big")
                    for ko in range(KO):
                        nc.tensor.matmul(ps[:], wq_sb[:, ko, j * P:(j + 1) * P],
                                         x_sb[:, ko, nt * 512:(nt + 1) * 512],
                                         start=(ko == 0), stop=(ko == KO - 1))
                    rope(qt_j[:, nt * 512:(nt + 1) * 512], ps, nt, f"q{j}")
                qts[j] = qt_j

            # ---- attention for one (qt, j) head-pair into ot tile ----
            def attn_block(qt, j, ot):
                pv0 = pv_ps.tile([65, 512], F32, tag="pv")
                pv1 = pv_ps.tile([65, 512], F32, tag="pv")
                nkb = 4 * qt + 4

                def flush_pv(prev):
                    # PV matmuls for the previous kb (software pipeline: issued
                    # after the next kb's scores so PE never waits on ACT's exp
                    # of the current block). Diagonal blocks only touch output
                    # columns >= their first causally-valid query.
                    pkb, c0, q0, q1 = prev
                    nc.tensor.matmul(pv0[:, c0:512], v_sb[:, pkb, 0:65],
                                     q0[:, c0:512],
                                     start=(pkb == 0), stop=(pkb == nkb - 1))
                    nc.tensor.matmul(pv1[:, c0:512], v_sb[:, pkb, 65:130],
                                     q1[:, c0:512],
                                     start=(pkb == 0), stop=(pkb == nkb - 1))

                pending = []
                for kb in range(nkb):
                    tk = slice(kb * P, (kb + 1) * P)
                    jr = kb - 4 * qt           # >= 0 on diagonal blocks
                    c0 = max(0, jr) * P        # first causally-valid column
                    tqs = slice(qt * 512 + c0, (qt + 1) * 512)
                    s0 = s_ps.tile([P, 512], F32, tag="s")
                    s1 = s_ps.tile([P, 512], F32, tag="s")
                    nc.tensor.matmul(s0[:, c0:512], kt[0:64, tk],
                                     qts[j][0:64, tqs], start=True, stop=True)
                    nc.tensor.matmul(s1[:, c0:512], kt[64:128, tk],
                                     qts[j][64:128, tqs], start=True, stop=True)
                    if len(pending) >= 2:
                        flush_pv(pending.pop(0))
                    p0 = pexp.tile([P, 512], BF16, tag="p")
                    p1 = pexp.tile([P, 512], BF16, tag="p")
                    nc.scalar.activation(p0[:, c0:512], s0[:, c0:512],
                                         AF.Exp, scale=SCALE)
                    nc.scalar.activation(p1[:, c0:512], s1[:, c0:512],
                                         AF.Exp, scale=SCALE)
                    if jr >= 0:
                        # triangle mask on the one partially-valid block
                        nc.vector.tensor_mul(p0[:, c0:c0 + P],
                                             p0[:, c0:c0 + P], tri_sb[:])
                        nc.vector.tensor_mul(p1[:, c0:c0 + P],
                                             p1[:, c0:c0 + P], tri_sb[:])
                    pending.append((kb, c0, p0, p1))
                for pr in pending:
                    flush_pv(pr)
                # fast pv release: stage numerator + sumexp row, then the
                # recip/broadcast/mul chain runs off the pv critical path
                for idx, pv in ((0, pv0), (1, pv1)):
                    osl = ot[idx * 64:(idx + 1) * 64, j, :]
                    nc.vector.tensor_copy(osl, pv[0:64, :])
                    srow = rrp.tile([1, 512], F32, tag="sr")
                    nc.vector.tensor_copy(srow[:], pv[64:65, :])
                    rrow = rrp.tile([1, 512], F32, tag="rr")
                    nc.vector.reciprocal_approx_fast(rrow[:], srow[:])
                    drx = dramp.tile([1, 512], F32, tag="drx")
                    nc.sync.dma_start(drx[:], rrow[:])
                    bc = rrp.tile([P, 512], F32, tag="bc")
                    bsl = bc[idx * 64:(idx + 1) * 64, :]
                    nc.sync.dma_start(bsl, drx[:].to_broadcast((64, 512)))
                    nc.vector.tensor_mul(osl, osl, bsl)

            # ---- Wo output projection for one 128-row slice of a q tile ----
            def wo_block(qt, tt, ot):
                r0 = qt * 512 + tt * P
                for oc in range(4):
                    yps = big_ps.tile([P, 512], F32, tag="big")
                    for kf in range(4):
                        nc.tensor.matmul(yps[:], ot[:, kf, tt * P:(tt + 1) * P],
                                         wo_sb[:, kf, oc * 512:(oc + 1) * 512],
                                         start=(kf == 0), stop=(kf == 3))
                    ysb = work.tile([P, 512], F32, tag="ysb")
                    nc.vector.tensor_copy(ysb[:], yps[:])
                    nc.sync.dma_start(y[r0:r0 + P, oc * 512:(oc + 1) * 512], ysb[:])

            # ---- emission order: interleave phases so PE work (proj / Wo)
            # fills the gaps while ACT runs exp of the attention stream ----
            ot_tiles = {0: otp.tile([P, 4, 512], BF16, tag="ot", name="ot0")}
            for j in range(4):
                q_proj(j)
                if j == 1:
                    for kf in range(4):
                        nc.sync.dma_start(wo_sb[:, kf, :], wo[kf * P:(kf + 1) * P, :])
                attn_block(0, j, ot_tiles[0])      # qt=0 rides inside Q proj
            for qt in range(1, NT):
                ot_tiles[qt] = otp.tile([P, 4, 512], BF16, tag="ot", name=f"ot{qt}")
                for j in range(4):
                    attn_block(qt, j, ot_tiles[qt])
                    wo_block(qt - 1, j, ot_tiles[qt - 1])  # prev qt's Wo rides along
            for tt in range(4):
                wo_block(NT - 1, tt, ot_tiles[NT - 1])

    nc.finalize()
    _nc_cache["nc"] = nc
    return nc


def make_in_maps(x, Wq, Wk, Wv, Wo):
    bf = ml_dtypes.bfloat16
    x = np.asarray(x, np.float32)
    Wq = np.asarray(Wq, np.float32)
    Wk = np.asarray(Wk, np.float32)
    Wv = np.asarray(Wv, np.float32)
    Wo = np.asarray(Wo, np.float32)

    # rope tables, [128, T]: row p covers head-dim d = p % 64
    half = HD // 2
    inv_freq = 1.0 / (ROPE_BASE ** (np.arange(half, dtype=np.float64) / half))
    pos = np.arange(T, dtype=np.float64)
    d_idx = np.arange(P) % HD
    freqs = pos[None, :] * inv_freq[d_idx % half][:, None]      # [128, T]
    cos_t = np.cos(freqs).astype(bf)
    sign = np.where(d_idx < half, -1.0, 1.0)[:, None]
    sin_t = (np.sin(freqs) * sign).astype(bf)

    # causal 0/1 triangle for the partially-valid diagonal sub-block
    pp = np.arange(P)[:, None]
    ff = np.arange(P)[None, :]
    tri = (ff >= pp).astype(bf)

    in_maps = []
    for c in range(8):
        b, g = c // 4, c % 4
        heads = [8 * g + h for h in PERM_Q]
        qrows = np.concatenate([np.arange(h * HD, (h + 1) * HD) for h in heads])
        kvrows = np.arange(2 * g * HD, (2 * g + 2) * HD)
        in_maps.append({
            "xT": np.ascontiguousarray(x[b].T).astype(bf),
            "wqT": np.ascontiguousarray(Wq[qrows, :].T).astype(bf),
            "wkT": np.ascontiguousarray(Wk[kvrows, :].T).astype(bf),
            "wvT": np.ascontiguousarray(Wv[kvrows, :].T).astype(bf),
            "woT": np.ascontiguousarray(Wo[:, qrows].T).astype(bf),
            "cosT": cos_t,
            "sinT": sin_t,
            "tri": tri,
        })
    return in_maps


def combine_outputs(results):
    out = np.zeros((B, T, D), np.float32)
    for c in range(8):
        out[c // 4] += results[c]["y"]
    return out


def _ensure_ntff_hook():
    """Register the axon NTFF profile hook (antenv.axon_hooks is missing
    from this image; recreate it and wire the ctypes hook from trn_boot)."""
    import sys, types
    if "antenv.axon_hooks" in sys.modules:
        return
    m = types.ModuleType("antenv.axon_hooks")
    hook = [None]
    m.set_axon_ntff_profile_hook = lambda h: hook.__setitem__(0, h)
    m.get_axon_ntff_profile_hook = lambda: hook[0]
    sys.modules["antenv.axon_hooks"] = m
    import antenv
    antenv.axon_hooks = m
    sys.path.insert(0, "/root/.axon_site")
    from trn_agent_boot.trn_boot import _ntff_profile_via_ctypes
    m.set_axon_ntff_profile_hook(
        _ntff_profile_via_ctypes("/opt/axon/libaxon_pjrt.so"))


def kernel(x, Wq, Wk, Wv, Wo, _trace=False):
    if _trace:
        _ensure_ntff_hook()
    nc = build_nc()
    in_maps = make_in_maps(x, Wq, Wk, Wv, Wo)
    res = run_bass_kernel_spmd(nc, in_maps, core_ids=list(range(8)), trace=_trace)
    out = combine_outputs(res.results)
    if _trace:
        return out, res
    return out
